# revision 36
# baseline (speedup 1.0000x reference)
"""DualLSTM Trainium2 kernel (8-core SPMD), v2.

Strategy:
  - Embedding gather runs on host (pure indexing); x ships as bf16 [S, E].
  - gx = x @ W_ih^T (packed) as a tiled bf16 matmul on every core.
  - The 2047-step recurrence is replicated on all cores.  Per step the two
    matvecs u = W_hh @ h run with h stationary (lhsT [128,1]) and the packed
    bf16 weights streaming, as 4 column-tiled concurrent streams
    (tile_position=(0,32j)) accumulating into a [128,4,512] PSUM tile at
    partitions {0,32,64,96}.  Per-round DVE evicts + 4 affine scatter DMAs
    land u in a [128,64] gate-major tile; gates run as ~22 batched DVE ops +
    8 ACT ops (sigmoid/tanh share one table set).
  - fc1 replicated; fc2 (134-GFLOP vocab projection) sharded column-wise
    across the 8 cores; output returned bf16 and upcast on host.
"""

import os
from contextlib import ExitStack

import numpy as np
import ml_dtypes

import concourse.bass as bass
import concourse.tile as tile
import concourse.mybir as mybir
from concourse import bacc
from concourse.bass import ds, ts
from concourse.bass_utils import run_bass_kernel_spmd
from concourse.kernels.tile_matmul import matmul_tile_kernel

BF16 = ml_dtypes.bfloat16
F32 = mybir.dt.float32
BF = mybir.dt.bfloat16

V, E, H, S = 32000, 512, 1024, 2048
T = S - 1            # 2047 recurrence steps
TP = S
P = 128
HC = H // P          # 8 h-chunks
NCORES = 8
VS = V // NCORES     # 4000 real vocab columns per core
VSP = 4096           # padded vocab shard
NSTEPS = int(os.environ.get("DUAL_LSTM_STEPS", T))
NWARM = int(os.environ.get("DUAL_LSTM_WARM_MMS", 16))
NREPEAT = int(os.environ.get("DUAL_LSTM_REPEAT", 1))  # timing amplifier
# timing-diagnosis strip levels (break correctness, keep structure):
# 1=static gx, 2=+no scatter DMAs, 3=+no gate chain, 4=+no serial h dep
STRIP = int(os.environ.get("DUAL_LSTM_STRIP", 0))
SCAT = int(os.environ.get("DUAL_LSTM_SCAT", 2))  # 1 = whole-step 4-DMA scatter

AF = mybir.ActivationFunctionType
OP = mybir.AluOpType


def _pack_whh(W_en, W_cn):
    """[128, 2, 8, 8, 512] bf16: [p, cell, n, k, e] =
    W_cell[(n//2)*1024 + ((n%2)*4 + e%4)*128 + e//4, k*128 + p]
    (slot n//2 in original gate order i,f,g,o)"""
    Wb = np.stack([np.asarray(W_en, np.float32), np.asarray(W_cn, np.float32)])
    p = np.arange(P)[:, None, None, None, None]
    cell = np.arange(2)[None, :, None, None, None]
    n = np.arange(8)[None, None, :, None, None]
    k = np.arange(8)[None, None, None, :, None]
    e = np.arange(512)[None, None, None, None, :]
    row = (n // 2) * 1024 + ((n % 2) * 4 + e % 4) * 128 + e // 4
    return np.ascontiguousarray(Wb[cell, row, k * 128 + p]).astype(BF16)


def _pack_wih(W_en, W_cn):
    """[E, 8192] bf16.  SCAT<2: col p*64 + slot*16 + cell*8 + hc;
    SCAT=2: col p*64 + cell*32 + jj*16 + slot*4 + b (hc = jj*4+b).
    Both map to row W_ih_cell[slot*1024 + hc*128 + p, :]."""
    Wb = np.stack([np.asarray(W_en, np.float32), np.asarray(W_cn, np.float32)])
    cp = np.arange(2 * 4 * H)
    p, c = cp // 64, cp % 64
    if SCAT >= 2:
        cell, jj, sl, b = c // 32, (c % 32) // 16, (c % 16) // 4, c % 4
        hc = jj * 4 + b
    else:
        sl, cell, hc = c // 16, (c % 16) // 8, c % 8
    rows = sl * 1024 + hc * 128 + p
    return np.ascontiguousarray(Wb[cell, rows, :].T).astype(BF16)


def build(nsteps=NSTEPS, nrepeat=NREPEAT):
    nc = bacc.Bacc(None, target_bir_lowering=False, debug=False)

    # ---- kernel I/O ----
    x_d = nc.dram_tensor("x_d", [S, E], BF, kind="ExternalInput").ap()
    wih = nc.dram_tensor("wih", [E, 2 * 4 * H], BF, kind="ExternalInput").ap()
    whh = nc.dram_tensor("whh", [P, 2, 8, 8, 512], BF, kind="ExternalInput").ap()
    maskb = nc.dram_tensor("maskb", [1, T], F32, kind="ExternalInput").ap()
    w1t = nc.dram_tensor("w1t", [H, H], BF, kind="ExternalInput").ap()
    b1p = nc.dram_tensor("b1p", [P, HC], F32, kind="ExternalInput").ap()
    w2t = nc.dram_tensor("w2t", [H, VSP], BF, kind="ExternalInput").ap()
    b2v = nc.dram_tensor("b2v", [1, VSP], F32, kind="ExternalInput").ap()
    out = nc.dram_tensor("out", [TP, VSP], BF, kind="ExternalOutput").ap()

    # ---- DRAM intermediates ----
    gxs = nc.dram_tensor("gxs", [TP, 2 * 4 * H], BF).ap()
    outst = nc.dram_tensor("outst", [H, TP], BF).ap()
    hidt = nc.dram_tensor("hidt", [H, TP], BF).ap()

    # ============ phase B: gx[t, :] = x[t] @ wih ============
    with tile.TileContext(nc) as tc:
        matmul_tile_kernel(
            tc,
            kxm_ap=x_d,            # [S, E] -> transposed to [E, S]
            kxn_ap=wih,            # [E, 8192]
            mxn_ap=gxs,            # [2048, 8192]
            transpose_kxm=True,
        )

    # ================= phase C: recurrence =================
    with tile.TileContext(nc) as tc:
        cr = ExitStack()
        with cr:
            wp = cr.enter_context(tc.tile_pool(name="wp", bufs=1))
            sp = cr.enter_context(tc.tile_pool(name="sp", bufs=1))
            gxp = cr.enter_context(tc.tile_pool(name="gxp", bufs=3))
            stp = cr.enter_context(tc.tile_pool(name="stp", bufs=1))
            drp = cr.enter_context(tc.tile_pool(name="drp", bufs=2, space="DRAM"))
            usp = cr.enter_context(tc.tile_pool(name="usp", bufs=2))
            ep = cr.enter_context(tc.tile_pool(name="ep", bufs=2))
            pp = cr.enter_context(tc.tile_pool(name="pp", bufs=1, space="PSUM"))

            whh_sb = wp.tile([P, 2, 8, 8, 512], BF)
            nc.sync.dma_start(whh_sb[:], whh)
            mrow = sp.tile([1, T], F32)
            nc.sync.dma_start(mrow[:], maskb)
            mask_sb = sp.tile([P, T], F32)
            nc.gpsimd.partition_broadcast(mask_sb[:], mrow[:])
            outs_sb = sp.tile([P, HC, TP], BF)
            nc.gpsimd.memset(outs_sb[:], 0.0)

            h_en = sp.tile([P, HC], BF)
            h_cn = sp.tile([P, HC], BF)
            c_st = sp.tile([P, HC], F32)
            nc.gpsimd.memset(h_en[:], 0.0)
            nc.gpsimd.memset(h_cn[:], 0.0)
            nc.gpsimd.memset(c_st[:], 0.0)

            # one PSUM buffer per round, allocated for the whole loop; memset
            # once so the evict may read the 124 partitions the col-tiled MMs
            # don't write
            ups = [pp.tile([P, 512], F32, tag=f"ups{r}", name=f"ups{r}")
                   for r in range(4)]
            for r in range(4):
                nc.vector.memset(ups[r][:], 0.0)
            # pre-load the sigmoid/tanh ACT table set so the loop body needs
            # no per-step InstLoadActFuncSet
            warm = sp.tile([P, 1], F32)
            nc.scalar.activation(warm[:], c_st[:, 0:1], AF.Sigmoid)
            # constant lhsT for the PE warm-keeper dummy matmuls
            hwarm = sp.tile([P, 1], BF)
            nc.gpsimd.memset(hwarm[:], 0.0)

            def step(tv, gx_blk=None, u=0):
                if gx_blk is not None:
                    gx_t = gx_blk[:, 64 * u:64 * (u + 1)]
                else:
                    gxt_ = gxp.tile([P, 64], BF, tag="gx")
                    if STRIP < 1:
                        nc.sync.dma_start(
                            gxt_[:], gxs[ds(tv, 1)][0].rearrange("(p c) -> p c", c=64))
                    else:
                        nc.vector.memset(gxt_[:], 0.0)
                    gx_t = gxt_[:]

                # per-round: 32 col-tiled MMs -> evict; rounds = gates in
                # order [i, f, g, o].  Rounds 0-2 scatter together after the
                # g-evict and the whole c-chain runs hidden under round 3's
                # MM stream; only round 3's scatter + o-gates are in the tail.
                u_sb = usp.tile([P, 64], F32, tag="usb")
                a_t = ep.tile([P, 64], F32, tag="a")
                saif = ep.tile([P, 32], F32, tag="saif")
                tga = ep.tile([P, 16], F32, tag="tga")
                sbif = ep.tile([P, 32], F32, tag="sbif")
                tgb = ep.tile([P, 16], F32, tag="tgb")
                sao = ep.tile([P, 16], F32, tag="sao")
                sbo = ep.tile([P, 16], F32, tag="sbo")
                t1 = ep.tile([P, 16], F32, tag="t1")
                c1 = ep.tile([P, 16], F32, tag="c1")      # branch order [A, B]
                th1 = ep.tile([P, 16], F32, tag="th1")
                t2 = ep.tile([P, 16], F32, tag="t2")
                c2 = ep.tile([P, 16], F32, tag="c2")      # cell order [en(B), cn(A)]
                th2 = ep.tile([P, 16], F32, tag="th2")
                htok = ep.tile([P, 16], F32, tag="htok")  # [hA_en, hB_cn]
                hdum = ep.tile([P, 16], F32, tag="hdum")  # [hB_en, hA_cn]
                dd = ep.tile([P, 3, 8], F32, tag="dd")
                mt = mask_sb[:, ds(tv, 1)]
                # staging: rounds 0-2 at per-partition offset p*12 + r*4 + b
                # (so each scatter source folds contiguous); round 3 plain
                if SCAT:
                    stg012 = stp.tile([P, 2048], F32, tag="stg012")
                    sv = stg012[:].rearrange("q (p r2 b) -> q p r2 b", r2=4, b=4)
                    stg3 = stg012
                else:
                    stg012 = stp.tile([P, 1536], F32, tag="stg012")
                    sv = stg012[:].rearrange("q (p r2 b) -> q p r2 b", r2=3, b=4)
                    stg3 = stp.tile([P, 512], F32, tag="stg3")
                uv = u_sb[:].rearrange("p (r j b) -> p r j b", r=4, j=4, b=4)

                for r in range(4):
                    for k in range(HC):
                        for j in range(4):
                            cell = j // 2
                            n = 2 * r + (j % 2)
                            hbuf = (h_en, h_cn)[cell]
                            nc.tensor.matmul(
                                ups[r][32 * j:32 * j + 1],
                                lhsT=hbuf[:, k:k + 1],
                                rhs=whh_sb[:, cell, n, k],
                                start=(k == 0), stop=(k == HC - 1),
                                tile_position=(0, 32 * j))
                    if SCAT or r < 3:
                        nc.vector.tensor_copy(
                            sv[:, :, r, :],
                            ups[r][:].rearrange("q (p b) -> q p b", b=4))
                    else:
                        nc.vector.tensor_copy(stg3[:], ups[3][:])
                    if r == 2 and not SCAT:
                        # scatter slots i, f, g for all 4 groups (two HWDGE
                        # queues + two SWDGE queues so they overlap)
                        if STRIP < 2:
                            for j, eng in enumerate((nc.sync, nc.scalar,
                                                     nc.gpsimd, nc.gpsimd)):
                                eng.dma_start(uv[:, 0:3, j, :],
                                              stg012[32 * j:32 * j + 1])
                        else:
                            nc.vector.memset(u_sb[:, 0:48], 0.01)
                        # gates for slots i, f, g + both c-chains (all of
                        # this overlaps round 3's MM stream)
                        if STRIP >= 3:
                            continue
                        nc.vector.tensor_tensor(out=a_t[:, 0:48], in0=u_sb[:, 0:48],
                                                in1=gx_t[:, 0:48], op=OP.add)
                        nc.scalar.activation(saif[:], a_t[:, 0:32], AF.Sigmoid)
                        nc.scalar.activation(tga[:], a_t[:, 32:48], AF.Tanh)
                        nc.scalar.activation(sbif[:], u_sb[:, 0:32], AF.Sigmoid)
                        nc.scalar.activation(tgb[:], u_sb[:, 32:48], AF.Tanh)
                        nc.vector.tensor_tensor(out=t1[:], in0=saif[:, 0:16], in1=tga[:], op=OP.mult)
                        nc.vector.tensor_tensor(out=c1[:, 0:8], in0=saif[:, 16:24], in1=c_st[:], op=OP.mult)
                        nc.vector.tensor_tensor(out=c1[:, 8:16], in0=saif[:, 24:32], in1=c_st[:], op=OP.mult)
                        nc.vector.tensor_tensor(out=c1[:], in0=c1[:], in1=t1[:], op=OP.add)
                        nc.scalar.activation(th1[:], c1[:], AF.Tanh)
                        nc.vector.tensor_tensor(out=t2[:], in0=sbif[:, 0:16], in1=tgb[:], op=OP.mult)
                        nc.vector.tensor_tensor(out=c2[:, 0:8], in0=sbif[:, 16:24], in1=c1[:, 8:16], op=OP.mult)
                        nc.vector.tensor_tensor(out=c2[:, 8:16], in0=sbif[:, 24:32], in1=c1[:, 0:8], op=OP.mult)
                        nc.vector.tensor_tensor(out=c2[:], in0=c2[:], in1=t2[:], op=OP.add)
                        nc.scalar.activation(th2[:], c2[:], AF.Tanh)

                # o-slot tail: scatter round 3, then the short select chain
                if STRIP < 2:
                    if SCAT >= 2:
                        # 2-DMA scatter via DRAM bounce: A) 4 partitions ->
                        # DRAM at offset p*64 + j*16 + (r*4+b); B) contiguous
                        # readback into u_sb col = j*16 + r*4 + b
                        bounce = drp.tile([P, 64], F32, tag="bounce")
                        nc.sync.dma_start(
                            bounce[:].rearrange("p (j rb) -> j p rb", j=4),
                            stg012[0:97:32])
                        nc.scalar.dma_start(u_sb[:], bounce[:])
                    elif SCAT:
                        for j, eng in enumerate((nc.sync, nc.scalar,
                                                 nc.gpsimd, nc.gpsimd)):
                            eng.dma_start(uv[:, :, j, :],
                                          stg012[32 * j:32 * j + 1])
                    else:
                        for j, eng in enumerate((nc.sync, nc.scalar,
                                                 nc.gpsimd, nc.gpsimd)):
                            eng.dma_start(uv[:, 3, j, :], stg3[32 * j:32 * j + 1])
                else:
                    nc.vector.memset(u_sb[:, 48:64], 0.01)
                if SCAT >= 2 and STRIP < 3:
                    nc.vector.tensor_tensor(out=a_t[:], in0=u_sb[:],
                                            in1=gx_t[:], op=OP.add)
                    a4 = a_t[:].rearrange("p (cj r b) -> p cj r b", cj=4, b=4)
                    u4 = u_sb[:].rearrange("p (cj r b) -> p cj r b", cj=4, b=4)
                    s_i = saif[:, 0:16].rearrange("p (cj b) -> p cj b", b=4)
                    s_f = saif[:, 16:32].rearrange("p (cj b) -> p cj b", b=4)
                    nc.scalar.activation(s_i, a4[:, :, 0, :], AF.Sigmoid)
                    nc.scalar.activation(s_f, a4[:, :, 1, :], AF.Sigmoid)
                    nc.scalar.activation(
                        tga[:].rearrange("p (cj b) -> p cj b", b=4),
                        a4[:, :, 2, :], AF.Tanh)
                    b_i = sbif[:, 0:16].rearrange("p (cj b) -> p cj b", b=4)
                    b_f = sbif[:, 16:32].rearrange("p (cj b) -> p cj b", b=4)
                    nc.scalar.activation(b_i, u4[:, :, 0, :], AF.Sigmoid)
                    nc.scalar.activation(b_f, u4[:, :, 1, :], AF.Sigmoid)
                    nc.scalar.activation(
                        tgb[:].rearrange("p (cj b) -> p cj b", b=4),
                        u4[:, :, 2, :], AF.Tanh)
                    nc.vector.tensor_tensor(out=t1[:], in0=saif[:, 0:16], in1=tga[:], op=OP.mult)
                    nc.vector.tensor_tensor(out=c1[:, 0:8], in0=saif[:, 16:24], in1=c_st[:], op=OP.mult)
                    nc.vector.tensor_tensor(out=c1[:, 8:16], in0=saif[:, 24:32], in1=c_st[:], op=OP.mult)
                    nc.vector.tensor_tensor(out=c1[:], in0=c1[:], in1=t1[:], op=OP.add)
                    nc.scalar.activation(th1[:], c1[:], AF.Tanh)
                    nc.vector.tensor_tensor(out=t2[:], in0=sbif[:, 0:16], in1=tgb[:], op=OP.mult)
                    nc.vector.tensor_tensor(out=c2[:, 0:8], in0=sbif[:, 16:24], in1=c1[:, 8:16], op=OP.mult)
                    nc.vector.tensor_tensor(out=c2[:, 8:16], in0=sbif[:, 24:32], in1=c1[:, 0:8], op=OP.mult)
                    nc.vector.tensor_tensor(out=c2[:], in0=c2[:], in1=t2[:], op=OP.add)
                    nc.scalar.activation(th2[:], c2[:], AF.Tanh)
                elif SCAT and STRIP < 3:
                    nc.vector.tensor_tensor(out=a_t[:, 0:48], in0=u_sb[:, 0:48],
                                            in1=gx_t[:, 0:48], op=OP.add)
                    nc.scalar.activation(saif[:], a_t[:, 0:32], AF.Sigmoid)
                    nc.scalar.activation(tga[:], a_t[:, 32:48], AF.Tanh)
                    nc.scalar.activation(sbif[:], u_sb[:, 0:32], AF.Sigmoid)
                    nc.scalar.activation(tgb[:], u_sb[:, 32:48], AF.Tanh)
                    nc.vector.tensor_tensor(out=t1[:], in0=saif[:, 0:16], in1=tga[:], op=OP.mult)
                    nc.vector.tensor_tensor(out=c1[:, 0:8], in0=saif[:, 16:24], in1=c_st[:], op=OP.mult)
                    nc.vector.tensor_tensor(out=c1[:, 8:16], in0=saif[:, 24:32], in1=c_st[:], op=OP.mult)
                    nc.vector.tensor_tensor(out=c1[:], in0=c1[:], in1=t1[:], op=OP.add)
                    nc.scalar.activation(th1[:], c1[:], AF.Tanh)
                    nc.vector.tensor_tensor(out=t2[:], in0=sbif[:, 0:16], in1=tgb[:], op=OP.mult)
                    nc.vector.tensor_tensor(out=c2[:, 0:8], in0=sbif[:, 16:24], in1=c1[:, 8:16], op=OP.mult)
                    nc.vector.tensor_tensor(out=c2[:, 8:16], in0=sbif[:, 24:32], in1=c1[:, 0:8], op=OP.mult)
                    nc.vector.tensor_tensor(out=c2[:], in0=c2[:], in1=t2[:], op=OP.add)
                    nc.scalar.activation(th2[:], c2[:], AF.Tanh)
                # keep the PE p-state warm through the tail: dummy MMs with
                # no data deps, overwritten by next step's start=True
                for dmy in range(NWARM):
                    nc.tensor.matmul(
                        ups[0][0:1], lhsT=hwarm[:, 0:1],
                        rhs=whh_sb[:, 0, 0, 0], start=True, stop=True,
                        tile_position=(0, 0), skip_group_check=True)
                if STRIP >= 3:
                    if STRIP >= 4:
                        nc.vector.tensor_copy(h_en[:], whh_sb[:, 0, 0, 0, 0:8])
                        nc.vector.tensor_copy(h_cn[:], whh_sb[:, 0, 0, 0, 8:16])
                    else:
                        nc.vector.tensor_copy(h_en[:], stg3[:, 0:8])
                        nc.vector.tensor_copy(h_cn[:], stg3[:, 8:16])
                    return
                if SCAT >= 2:
                    a4o = a_t[:].rearrange("p (cj r b) -> p cj r b", cj=4, b=4)
                    u4o = u_sb[:].rearrange("p (cj r b) -> p cj r b", cj=4, b=4)
                    nc.scalar.activation(
                        sao[:].rearrange("p (cj b) -> p cj b", b=4),
                        a4o[:, :, 3, :], AF.Sigmoid)
                    nc.scalar.activation(
                        sbo[:].rearrange("p (cj b) -> p cj b", b=4),
                        u4o[:, :, 3, :], AF.Sigmoid)
                else:
                    nc.vector.tensor_tensor(out=a_t[:, 48:64], in0=u_sb[:, 48:64],
                                            in1=gx_t[:, 48:64], op=OP.add)
                    nc.scalar.activation(sao[:], a_t[:, 48:64], AF.Sigmoid)
                    nc.scalar.activation(sbo[:], u_sb[:, 48:64], AF.Sigmoid)
                nc.vector.tensor_tensor(out=htok[:], in0=sao[:], in1=th1[:], op=OP.mult)
                nc.vector.tensor_tensor(out=hdum[:], in0=sbo[:], in1=th2[:], op=OP.mult)
                # h_en / h_cn first: they gate the next step's MMs
                nc.vector.tensor_tensor(out=dd[:, 0], in0=htok[:, 0:8], in1=hdum[:, 0:8], op=OP.subtract)
                nc.vector.scalar_tensor_tensor(
                    out=h_en[:], in0=dd[:, 0], scalar=mt, in1=hdum[:, 0:8],
                    op0=OP.mult, op1=OP.add)
                nc.vector.tensor_tensor(out=dd[:, 1], in0=hdum[:, 8:16], in1=htok[:, 8:16], op=OP.subtract)
                nc.vector.scalar_tensor_tensor(
                    out=h_cn[:], in0=dd[:, 1], scalar=mt, in1=htok[:, 8:16],
                    op0=OP.mult, op1=OP.add)
                nc.vector.tensor_tensor(out=dd[:, 2], in0=c2[:, 8:16], in1=c2[:, 0:8], op=OP.subtract)
                nc.vector.scalar_tensor_tensor(
                    out=c_st[:], in0=dd[:, 2], scalar=mt, in1=c2[:, 0:8],
                    op0=OP.mult, op1=OP.add)
                nc.vector.tensor_tensor(
                    out=outs_sb[:, :, ds(tv, 1)].rearrange("p k o -> p (k o)"),
                    in0=h_en[:], in1=h_cn[:], op=OP.add)

            for rep in range(nrepeat):
                if rep:
                    nc.gpsimd.memset(h_en[:], 0.0)
                    nc.gpsimd.memset(h_cn[:], 0.0)
                    nc.gpsimd.memset(c_st[:], 0.0)
                UNR = 4
                if nsteps > UNR:
                    main = (nsteps // UNR) * UNR
                    with tc.For_i(0, main, UNR,
                                  hint_engines=(mybir.EngineType.PE,)) as iv:
                        gx_blk = gxp.tile([P, UNR * 64], BF, tag="gxblk")
                        if STRIP < 1:
                            nc.sync.dma_start(
                                gx_blk[:],
                                gxs[ds(iv, UNR)].rearrange(
                                    "t (p c) -> p t c", c=64))
                        else:
                            nc.vector.memset(gx_blk[:], 0.0)
                        for u in range(UNR):
                            step(iv + u, gx_blk, u)
                    for t_ in range(main, nsteps):
                        step(t_)
                else:
                    for t_ in range(nsteps):
                        step(t_)

            nc.sync.dma_start(outst.rearrange("(k p) t -> p k t", p=P), outs_sb[:])

    # ============ phase D: hidT = relu(w1 @ outsT + b1) ============
    with tile.TileContext(nc) as tc:
        with ExitStack() as c3:
            bp = c3.enter_context(tc.tile_pool(name="bias1", bufs=1))
            b1_sb = bp.tile([P, HC], F32)
            nc.sync.dma_start(b1_sb[:], b1p)

            def relu_bias(nc_, psum, sbuf, md):
                mabs = md.m_tile_idx * md.m_subtiles + md.m_subtile_idx
                nc_.scalar.activation(sbuf[:], psum[:], AF.Relu,
                                      bias=b1_sb[:, mabs:mabs + 1])

            from concourse.kernels.tile_matmul import (
                composable_matmul_tile_kernel, dma_from_dram_kxm,
                dma_from_dram_kxn, dma_to_dram_mxn)
            kxm_pool = c3.enter_context(tc.tile_pool(name="kxm1", bufs=3))
            kxn_pool = c3.enter_context(tc.tile_pool(name="kxn1", bufs=3))
            kxm_producer, kxm_shape = dma_from_dram_kxm(kxm_pool, w1t)
            kxn_producer, kxn_shape = dma_from_dram_kxn(kxn_pool, outst)
            composable_matmul_tile_kernel(
                tc, kxm_shape, kxn_shape, hidt.dtype,
                kxm_producer, kxn_producer,
                mxn_consumer=dma_to_dram_mxn(hidt),
                mxn_subtile_reducer=relu_bias)

    # ============ phase E: out = hidT.T @ w2T + b2 ============
    with tile.TileContext(nc) as tc:
        with ExitStack() as c4:
            bp2 = c4.enter_context(tc.tile_pool(name="bias2", bufs=1))
            b2row = bp2.tile([1, VSP], F32)
            nc.sync.dma_start(b2row[:], b2v)
            b2_sb = bp2.tile([P, VSP], F32)
            nc.gpsimd.partition_broadcast(b2_sb[:], b2row[:])

            def add_b2(nc_, sbuf, md, _):
                for si in range(sbuf.shape[1]):
                    nc_.vector.tensor_tensor(
                        out=sbuf[:, si, :], in0=sbuf[:, si, :],
                        in1=b2_sb[:, md.n_slice], op=OP.add)

            matmul_tile_kernel(
                tc,
                kxm_ap=hidt,          # [H, TP]
                kxn_ap=w2t,           # [H, VSP]
                mxn_ap=out,           # [TP, VSP] bf16
                post_mxn_tile_fn=add_b2,
            )

    nc.compile()
    return nc


_CACHE = {}


def _get_nc(nsteps=NSTEPS):
    if nsteps not in _CACHE:
        _CACHE[nsteps] = build(nsteps)
    return _CACHE[nsteps]


def prep_in_maps(sentence, mask, embedding, W_ih_en, W_hh_en, W_ih_cn, W_hh_cn,
                 fc_w1, fc_b1, fc_w2, fc_b2):
    sent = np.asarray(sentence).astype(np.int64)
    emb = np.asarray(embedding, np.float32)
    x = np.zeros((S, E), np.float32)
    x[:T] = emb[sent[:-1]]

    common = {
        "x_d": x.astype(BF16),
        "whh": _pack_whh(W_hh_en, W_hh_cn),
        "wih": _pack_wih(W_ih_en, W_ih_cn),
        "maskb": np.asarray(mask, np.float32).reshape(1, T),
        "w1t": np.ascontiguousarray(np.asarray(fc_w1, np.float32).T).astype(BF16),
        "b1p": np.asarray(fc_b1, np.float32).reshape(HC, P).T.copy(),
    }
    in_maps = []
    for i in range(NCORES):
        w2s = np.zeros((H, VSP), BF16)
        w2s[:, :VS] = np.asarray(fc_w2, np.float32)[i * VS:(i + 1) * VS].T.astype(BF16)
        b2s = np.zeros((1, VSP), np.float32)
        b2s[0, :VS] = np.asarray(fc_b2, np.float32)[i * VS:(i + 1) * VS]
        in_maps.append({**common, "w2t": w2s, "b2v": b2s})
    return in_maps


def kernel(**inputs):
    in_maps = prep_in_maps(**inputs)
    nc = _get_nc()
    res = run_bass_kernel_spmd(nc, in_maps, list(range(NCORES)))
    return np.concatenate(
        [r["out"][:T, :VS].astype(np.float32) for r in res.results], axis=1)
